# revision 1
# baseline (speedup 1.0000x reference)
"""GQA attention forward on 8 TRN2 NeuronCores, tensor-parallel across heads.

Problem (hardcoded): B=2, T=2048, D=2048, 16 q-heads, 4 kv-heads, head_dim=128,
RoPE (rotate-half pairing i <-> i+64), causal softmax, output projection.

Sharding (per core c of 8):
  q-heads 2c, 2c+1 (rows 256c:256c+256 of wq), kv-head c//2 (rows of wk/wv),
  wo input-dim slice [:, 256c:256c+256]. x replicated. Each core computes a
  full-shape partial of the output (y_local @ wo_slice.T); host sums partials.

On-core layout: activations kept feature-major (qT/kT = [head_dim, tokens]) so
every matmul contraction dim lands on SBUF partitions with zero transposes of
x (host pre-transposes x once). Scores are computed transposed (ST[j,i]) so
P@V needs no transpose either; softmax denominators come from a ones-vector
matmul; max-subtraction is skipped (scores are O(5), exp is safe in fp32).
All matmuls run in float32r (fp32 bits, relaxed PE mode: full speed at N>=256,
~1e-4 matmul rel err).
"""
import math
import numpy as np

P = 128
B = 2
T = 2048
D = 2048
BT = B * T            # 4096
HD = 128              # head dim
QH = 2                # local q heads per core
KT = D // P           # 16 contraction tiles over D
NB = 512              # free-dim block (tokens) for matmuls
NBLK = BT // NB       # 8 bt blocks
IB = T // NB          # 4 i-blocks per batch
NCORES = 8
SCALE = 1.0 / math.sqrt(HD)

_CACHE = {}


def _build():
    import concourse.bass as bass
    import concourse.mybir as mybir
    from concourse import bacc
    from concourse.tile import TileContext

    F32 = mybir.dt.float32
    F32R = mybir.dt.float32r
    EXP = mybir.ActivationFunctionType.Exp

    nc = bacc.Bacc("TRN2", target_bir_lowering=False, debug=False)

    xT_d = nc.dram_tensor("xT", [D, BT], F32R, kind="ExternalInput").ap()
    wqT_d = nc.dram_tensor("wqT", [D, QH * HD], F32R, kind="ExternalInput").ap()
    wkT_d = nc.dram_tensor("wkT", [D, HD], F32R, kind="ExternalInput").ap()
    wvT_d = nc.dram_tensor("wvT", [D, HD], F32R, kind="ExternalInput").ap()
    woT_d = nc.dram_tensor("woT", [QH * HD, D], F32R, kind="ExternalInput").ap()
    cosT_d = nc.dram_tensor("cosT", [P, T], F32R, kind="ExternalInput").ap()
    ssinT_d = nc.dram_tensor("ssinT", [P, T], F32R, kind="ExternalInput").ap()
    permT_d = nc.dram_tensor("permT", [P, P], F32R, kind="ExternalInput").ap()
    triu_d = nc.dram_tensor("triu", [P, P], F32R, kind="ExternalInput").ap()
    ident_d = nc.dram_tensor("ident", [P, P], F32R, kind="ExternalInput").ap()
    onesv_d = nc.dram_tensor("onesv", [P, 1], F32R, kind="ExternalInput").ap()
    out_d = nc.dram_tensor("out", [BT, D], F32, kind="ExternalOutput").ap()

    with TileContext(nc) as tc:
        with (
            tc.tile_pool(name="consts", bufs=1) as consts,
            tc.tile_pool(name="acts", bufs=1) as acts,
        ):
            # ---- resident constants / weights ----
            wq_sb = consts.tile([P, KT, QH * HD], F32R)
            wk_sb = consts.tile([P, KT, HD], F32R)
            wv_sb = consts.tile([P, KT, HD], F32R)
            wo_sb = consts.tile([P, QH, D], F32R)
            cos_sb = consts.tile([P, T], F32R)
            sin_sb = consts.tile([P, T], F32R)
            perm_sb = consts.tile([P, P], F32R)
            triu_sb = consts.tile([P, P], F32R)
            id_sb = consts.tile([P, P], F32R)
            ones_sb = consts.tile([P, 1], F32R)
            nc.sync.dma_start(wq_sb, wqT_d.rearrange("(a p) m -> p a m", p=P))
            nc.sync.dma_start(wk_sb, wkT_d.rearrange("(a p) m -> p a m", p=P))
            nc.sync.dma_start(wv_sb, wvT_d.rearrange("(a p) m -> p a m", p=P))
            nc.sync.dma_start(wo_sb, woT_d.rearrange("(h p) j -> p h j", p=P))
            nc.sync.dma_start(cos_sb, cosT_d)
            nc.sync.dma_start(sin_sb, ssinT_d)
            nc.sync.dma_start(perm_sb, permT_d)
            nc.sync.dma_start(triu_sb, triu_d)
            nc.sync.dma_start(id_sb, ident_d)
            nc.sync.dma_start(ones_sb, onesv_d)

            # ---- resident activations ----
            qr_sb = acts.tile([P, QH, BT], F32R)   # roped qT
            kr_sb = acts.tile([P, BT], F32R)       # roped kT
            vt_sb = acts.tile([P, BT // P, HD], F32R)  # v token-major

            # ================= phase 1: projections + rope =================
            with (
                tc.tile_pool(name="xt", bufs=4) as xt_pool,
                tc.tile_pool(name="raw", bufs=5) as raw_pool,
                tc.tile_pool(name="ropew", bufs=4) as rope_pool,
                tc.tile_pool(name="pj", bufs=6, space="PSUM") as pj,
                tc.tile_pool(name="pperm", bufs=1, space="PSUM") as pperm,
                tc.tile_pool(name="ptr", bufs=1, space="PSUM") as ptr,
            ):
                for blk in range(NBLK):
                    c0 = blk * NB          # bt column base
                    t0 = (blk % IB) * NB   # rope table base (t = bt mod T)
                    ps_q0 = pj.tile([P, NB], F32, tag="pj")
                    ps_q1 = pj.tile([P, NB], F32, tag="pj")
                    ps_k = pj.tile([P, NB], F32, tag="pj")
                    ps_v = pj.tile([P, NB], F32, tag="pj")
                    for kt in range(KT):
                        xt = xt_pool.tile([P, NB], F32R, tag="xt")
                        nc.sync.dma_start(
                            xt, xT_d[kt * P:(kt + 1) * P, c0:c0 + NB]
                        )
                        st = kt == 0
                        sp = kt == KT - 1
                        nc.tensor.matmul(ps_q0, wq_sb[:, kt, 0:P], xt, start=st, stop=sp)
                        nc.tensor.matmul(ps_q1, wq_sb[:, kt, P:2 * P], xt, start=st, stop=sp)
                        nc.tensor.matmul(ps_k, wk_sb[:, kt, :], xt, start=st, stop=sp)
                        nc.tensor.matmul(ps_v, wv_sb[:, kt, :], xt, start=st, stop=sp)

                    # rope for q0, q1, k: roped = raw*cos + swap(raw)*ssin
                    for ps_raw, dst in (
                        (ps_q0, qr_sb[:, 0, c0:c0 + NB]),
                        (ps_q1, qr_sb[:, 1, c0:c0 + NB]),
                        (ps_k, kr_sb[:, c0:c0 + NB]),
                    ):
                        raw = raw_pool.tile([P, NB], F32R, tag="raw")
                        nc.scalar.copy(raw, ps_raw)
                        ps_sw = pperm.tile([P, NB], F32, tag="sw")
                        nc.tensor.matmul(ps_sw, perm_sb, raw, start=True, stop=True)
                        t1 = rope_pool.tile([P, NB], F32R, tag="t1")
                        nc.gpsimd.tensor_mul(t1, raw, cos_sb[:, t0:t0 + NB])
                        t2 = rope_pool.tile([P, NB], F32R, tag="t2")
                        nc.vector.tensor_mul(t2, ps_sw, sin_sb[:, t0:t0 + NB])
                        nc.vector.tensor_add(dst, t1, t2)

                    # v: drain feature-major, then PE-transpose to token-major
                    vraw = raw_pool.tile([P, NB], F32R, tag="raw")
                    nc.scalar.copy(vraw, ps_v)
                    for s in range(NB // P):
                        ps_t = ptr.tile([P, P], F32R, tag="tr")
                        nc.tensor.transpose(ps_t, vraw[:, s * P:(s + 1) * P], id_sb)
                        nc.vector.tensor_copy(vt_sb[:, blk * (NB // P) + s, :], ps_t)

            # ================= phase 2: attention + out-proj =================
            # Out-proj runs one i-block behind attention so the softmax
            # normalize chain (recip -> broadcast -> mult) never stalls the
            # PE queue: while it drains, the PE works on the next i-block's
            # score matmuls.
            with (
                tc.tile_pool(name="est", bufs=5) as est_pool,
                tc.tile_pool(name="ysb", bufs=3) as y_pool,
                tc.tile_pool(name="nrm", bufs=3) as nrm_pool,
                tc.tile_pool(name="osb", bufs=5) as out_pool,
                tc.tile_pool(name="pst", bufs=2, space="PSUM") as pst,
                tc.tile_pool(name="py", bufs=1, space="PSUM") as py,
                tc.tile_pool(name="pd", bufs=1, space="PSUM") as pd,
                tc.tile_pool(name="po", bufs=2, space="PSUM") as po,
            ):
                def emit_outproj(i0p, y_prev):
                    for s in range(NB // P):
                        row0 = i0p + s * P
                        for jb in range(D // NB):
                            ps_o = po.tile([P, NB], F32, tag="po")
                            for h2 in range(QH):
                                nc.tensor.matmul(
                                    ps_o,
                                    y_prev[:, h2, s * P:(s + 1) * P],
                                    wo_sb[:, h2, jb * NB:(jb + 1) * NB],
                                    start=(h2 == 0),
                                    stop=(h2 == QH - 1),
                                )
                            o_sb = out_pool.tile([P, NB], F32, tag="o")
                            nc.vector.tensor_copy(o_sb, ps_o)
                            nc.sync.dma_start(
                                out_d[row0:row0 + P, jb * NB:(jb + 1) * NB], o_sb
                            )

                pending = None
                for b in range(B):
                    cb = b * T  # bt base of this batch
                    for ib in range(IB):
                        i0 = cb + ib * NB  # global bt col base of q block
                        jt_max = 4 * ib + 3
                        y_sb = y_pool.tile([P, QH, NB], F32R, tag="y")
                        for h in range(QH):
                            ps_y = py.tile([P, NB], F32, tag="py")
                            ps_d = pd.tile([1, NB], F32, tag="pd")
                            for g in range(2 * ib + 2):  # groups of 2 j-tiles
                                ps_st = pst.tile([P, 2, NB], F32, tag="st")
                                est = est_pool.tile([P, 2, NB], F32R, tag="est")
                                subs = []
                                for u in range(2):
                                    jt = 2 * g + u
                                    a = jt - 4 * ib
                                    sub = max(0, a) * P
                                    subs.append(sub)
                                    nc.tensor.matmul(
                                        ps_st[:, u, sub:],
                                        kr_sb[:, cb + jt * P:cb + (jt + 1) * P],
                                        qr_sb[:, h, i0 + sub:i0 + NB],
                                        start=True,
                                        stop=True,
                                    )
                                if subs[0] == 0 and subs[1] == 0:
                                    nc.scalar.activation(est, ps_st, EXP, scale=SCALE)
                                else:
                                    for u in range(2):
                                        nc.scalar.activation(
                                            est[:, u, subs[u]:],
                                            ps_st[:, u, subs[u]:],
                                            EXP,
                                            scale=SCALE,
                                        )
                                for u in range(2):
                                    jt = 2 * g + u
                                    sub = subs[u]
                                    if jt >= 4 * ib:  # diagonal tile: tri mask
                                        nc.vector.tensor_mul(
                                            est[:, u, sub:sub + P],
                                            est[:, u, sub:sub + P],
                                            triu_sb,
                                        )
                                    st_f = jt == 0
                                    sp_f = jt == jt_max
                                    nc.tensor.matmul(
                                        ps_d[:, sub:],
                                        ones_sb,
                                        est[:, u, sub:],
                                        start=st_f,
                                        stop=sp_f,
                                    )
                                    nc.tensor.matmul(
                                        ps_y[:, sub:],
                                        vt_sb[:, (cb // P) + jt, :],
                                        est[:, u, sub:],
                                        start=st_f,
                                        stop=sp_f,
                                    )
                            r = nrm_pool.tile([1, NB], F32, tag="r")
                            nc.vector.reciprocal(r, ps_d)
                            rb = nrm_pool.tile([P, NB], F32, tag="rb")
                            nc.gpsimd.partition_broadcast(rb, r)
                            nc.vector.tensor_mul(y_sb[:, h, :], ps_y, rb)

                        if pending is not None:
                            emit_outproj(*pending)
                        pending = (i0, y_sb)
                emit_outproj(*pending)

    nc.compile()
    return nc


def _host_prep(x, rope, wq, wk, wv, wo):
    """Build the 8 per-core input maps (shard + pre-transpose on host)."""
    xT = np.ascontiguousarray(x.reshape(BT, D).T).astype(np.float32)
    cos = np.asarray(rope[..., 0], dtype=np.float32)  # [T, 64]
    sin = np.asarray(rope[..., 1], dtype=np.float32)
    cosT = np.ascontiguousarray(np.concatenate([cos.T, cos.T], axis=0))  # [128, T]
    ssinT = np.ascontiguousarray(np.concatenate([-sin.T, sin.T], axis=0))
    perm = np.zeros((P, P), dtype=np.float32)
    perm[(np.arange(P) + 64) % P, np.arange(P)] = 1.0
    triu = np.triu(np.ones((P, P), dtype=np.float32))
    ident = np.eye(P, dtype=np.float32)
    ones = np.ones((P, 1), dtype=np.float32)

    in_maps = []
    for c in range(NCORES):
        kv = c // 2
        in_maps.append(
            {
                "xT": xT,
                "wqT": np.ascontiguousarray(wq[QH * HD * c:QH * HD * (c + 1), :].T),
                "wkT": np.ascontiguousarray(wk[HD * kv:HD * (kv + 1), :].T),
                "wvT": np.ascontiguousarray(wv[HD * kv:HD * (kv + 1), :].T),
                "woT": np.ascontiguousarray(wo[:, QH * HD * c:QH * HD * (c + 1)].T),
                "cosT": cosT,
                "ssinT": ssinT,
                "permT": perm,
                "triu": triu,
                "ident": ident,
                "onesv": ones,
            }
        )
    return in_maps


LAST_RESULTS = None


def kernel(x, rope, wq, wk, wv, wo):
    global LAST_RESULTS
    from concourse import bass_utils

    if "nc" not in _CACHE:
        _CACHE["nc"] = _build()
    nc = _CACHE["nc"]

    in_maps = _host_prep(
        np.asarray(x), np.asarray(rope), np.asarray(wq), np.asarray(wk), np.asarray(wv),
        np.asarray(wo)
    )
    res = bass_utils.run_bass_kernel_spmd(nc, in_maps, core_ids=list(range(NCORES)))
    LAST_RESULTS = res
    acc = np.zeros((BT, D), dtype=np.float64)
    for c in range(NCORES):
        acc += res.results[c]["out"]
    return acc.reshape(B, T, D).astype(np.float32)



# revision 2
# speedup vs baseline: 1.1276x; 1.1276x over previous
"""GQA attention forward on 8 TRN2 NeuronCores, tensor-parallel across heads.

Problem (hardcoded): B=2, T=2048, D=2048, 16 q-heads, 4 kv-heads, head_dim=128,
RoPE (rotate-half pairing i <-> i+64), causal softmax, output projection.

Sharding (per core c of 8):
  q-heads 2c, 2c+1 (rows 256c:256c+256 of wq), kv-head c//2 (rows of wk/wv),
  wo input-dim slice [:, 256c:256c+256]. x replicated. Each core computes a
  full-shape partial of the output (y_local @ wo_slice.T); host sums partials.

v2 design vs the fp32r baseline:
  - bf16 activations/weights everywhere (f32 PSUM accumulate): halves DMA
    payloads, LDWEIGHTS size, and DVE element costs. CPU sim rel err 5.6e-3.
  - Single fused pipeline: proj(b,ib) -> attn(b,ib) -> outproj trails one
    i-block, with outproj matmuls emitted inside the next proj as PE filler
    during rope/psum drains.
  - Softmax denominator: est tiles accumulated elementwise on DVE into acc,
    one gpsimd partition_all_reduce (reduce+broadcast in one Pool op), then
    reciprocal_approx_fast (DVE custom op, ~5x faster than exact reciprocal).
    Removes all denominator matmuls from the PE (~66us) and the 4us exact
    reciprocals.
  - DMA: one 1MB load per 512-token x block (8 total), one store per
    128-token output row chunk (32 total), weights/consts on the ACT HWDGE
    queue so x loads start immediately on the SP queue.
  - PSUM budget exactly 8 banks: pj(2) + pst(2) + py(2) + po(2), with the
    rope-swap matmuls and v transposes sharing the pj ring.
"""
import math
import numpy as np

P = 128
B = 2
T = 2048
D = 2048
BT = B * T            # 4096
HD = 128              # head dim
QH = 2                # local q heads per core
KT = D // P           # 16 contraction tiles over D
NB = 512              # free-dim block (tokens)
IB = T // NB          # 4 i-blocks per batch
NJT_MAX = T // P      # 16 j-tiles per batch
NCORES = 8
SCALE = 1.0 / math.sqrt(HD)

_CACHE = {}


def _build():
    import concourse.bass as bass
    import concourse.mybir as mybir
    from concourse import bacc
    from concourse.bass import bass_isa
    from concourse.tile import TileContext

    F32 = mybir.dt.float32
    BF16 = mybir.dt.bfloat16
    EXP = mybir.ActivationFunctionType.Exp

    nc = bacc.Bacc("TRN2", target_bir_lowering=False, debug=False)

    xT_d = nc.dram_tensor("xT", [D, BT], BF16, kind="ExternalInput").ap()
    wqT_d = nc.dram_tensor("wqT", [D, QH * HD], BF16, kind="ExternalInput").ap()
    wkT_d = nc.dram_tensor("wkT", [D, HD], BF16, kind="ExternalInput").ap()
    wvT_d = nc.dram_tensor("wvT", [D, HD], BF16, kind="ExternalInput").ap()
    woT_d = nc.dram_tensor("woT", [QH * HD, D], BF16, kind="ExternalInput").ap()
    cb_d = nc.dram_tensor("cb", [P, 3 * P], BF16, kind="ExternalInput").ap()
    cf_d = nc.dram_tensor("cf", [P, 2 * T], F32, kind="ExternalInput").ap()
    out_d = nc.dram_tensor("out", [BT, D], BF16, kind="ExternalOutput").ap()

    with TileContext(nc) as tc:
        with (
            tc.tile_pool(name="consts", bufs=1) as consts,
            tc.tile_pool(name="acts", bufs=1) as acts,
            tc.tile_pool(name="xt", bufs=2) as xt_pool,
            tc.tile_pool(name="qr", bufs=2) as qr_pool,
            tc.tile_pool(name="raw", bufs=3) as raw_pool,
            tc.tile_pool(name="tt", bufs=2) as t_pool,
            tc.tile_pool(name="est", bufs=3) as est_pool,
            tc.tile_pool(name="accp", bufs=2) as acc_pool,
            tc.tile_pool(name="rinv", bufs=2) as rinv_pool,
            tc.tile_pool(name="ysb", bufs=2) as y_pool,
            tc.tile_pool(name="osb", bufs=2) as o_pool,
            tc.tile_pool(name="pj", bufs=2, space="PSUM") as pj,
            tc.tile_pool(name="pst", bufs=2, space="PSUM") as pst,
            tc.tile_pool(name="py", bufs=2, space="PSUM") as py,
            tc.tile_pool(name="po", bufs=1, space="PSUM") as po,
        ):
            # ---- resident constants / weights (ACT HWDGE queue) ----
            cb_sb = consts.tile([P, 3, P], BF16)
            wq_sb = consts.tile([P, KT, QH * HD], BF16)
            wk_sb = consts.tile([P, KT, HD], BF16)
            wv_sb = consts.tile([P, KT, HD], BF16)
            cs_sb = consts.tile([P, 2, T], F32)
            wo_sb = consts.tile([P, QH, D], BF16)
            nc.scalar.dma_start(cb_sb, cb_d.rearrange("p (a q) -> p a q", a=3))
            nc.scalar.dma_start(wq_sb, wqT_d.rearrange("(a p) m -> p a m", p=P))
            nc.scalar.dma_start(wk_sb, wkT_d.rearrange("(a p) m -> p a m", p=P))
            nc.scalar.dma_start(wv_sb, wvT_d.rearrange("(a p) m -> p a m", p=P))
            nc.scalar.dma_start(cs_sb, cf_d.rearrange("p (a t) -> p a t", a=2))
            nc.scalar.dma_start(wo_sb, woT_d.rearrange("(h p) j -> p h j", p=P))
            perm = cb_sb[:, 0, :]
            triu = cb_sb[:, 1, :]
            ident = cb_sb[:, 2, :]
            cos_t = cs_sb[:, 0, :]
            sin_t = cs_sb[:, 1, :]

            # ---- resident activations (per-batch slots) ----
            kr_sb = acts.tile([P, B, T], BF16)
            vt_sb = acts.tile([P, B, NJT_MAX, HD], BF16)

            xT_r = xT_d.rearrange("(a p) m -> p a m", p=P)
            xt_tiles = {}

            def prefetch(gblk):
                if gblk >= B * IB or gblk in xt_tiles:
                    return
                b, ib = divmod(gblk, IB)
                xt = xt_pool.tile([P, KT, NB], BF16, tag="xt", name="xt")
                c0 = b * T + ib * NB
                nc.sync.dma_start(xt, xT_r[:, :, c0:c0 + NB])
                xt_tiles[gblk] = xt

            def rope(ps_raw, dst, t0):
                # dst(bf16) = raw*cos + swap(raw)*ssin; swap via PE perm matmul
                raw = raw_pool.tile([P, NB], BF16, tag="raw")
                nc.scalar.copy(raw, ps_raw)  # frees the psum bank quickly
                t1 = t_pool.tile([P, NB], F32, tag="t1")
                nc.gpsimd.tensor_mul(t1, raw, cos_t[:, t0:t0 + NB])
                ps_sw = pj.tile([P, NB], F32, tag="pj", name="ps_sw")
                nc.tensor.matmul(ps_sw, perm, raw, start=True, stop=True)
                t2 = t_pool.tile([P, NB], F32, tag="t2")
                nc.vector.tensor_mul(t2, ps_sw, sin_t[:, t0:t0 + NB])
                nc.vector.tensor_add(dst, t1, t2)

            pending = [None]

            def emit_outproj_half(half):
                if pending[0] is None:
                    return
                i0p, y_prev, po_t = pending[0]
                for s in (0, 1) if half == 0 else (2, 3):
                    row0 = i0p + s * P
                    o_sb = o_pool.tile([P, D], BF16, tag="o", name="o_sb")
                    for jb in range(D // NB):
                        u = jb % 2
                        nc.tensor.matmul(
                            po_t[:, u, :],
                            y_prev[:, 0, s * P:(s + 1) * P],
                            wo_sb[:, 0, jb * NB:(jb + 1) * NB],
                            start=True,
                            stop=False,
                        )
                        nc.tensor.matmul(
                            po_t[:, u, :],
                            y_prev[:, 1, s * P:(s + 1) * P],
                            wo_sb[:, 1, jb * NB:(jb + 1) * NB],
                            start=False,
                            stop=True,
                        )
                        dst = o_sb[:, jb * NB:(jb + 1) * NB]
                        if u == 0:
                            nc.vector.tensor_copy(dst, po_t[:, u, :])
                        else:
                            nc.scalar.copy(dst, po_t[:, u, :])
                    nc.sync.dma_start(out_d[row0:row0 + P, :], o_sb)

            def emit_proj(b, ib, gblk):
                xt = xt_tiles.pop(gblk)
                prefetch(gblk + 2)
                t0 = ib * NB
                # pass A: the two local q heads
                ps_q0 = pj.tile([P, NB], F32, tag="pj", name="ps_q0")
                ps_q1 = pj.tile([P, NB], F32, tag="pj", name="ps_q1")
                for kt in range(KT):
                    st, sp = kt == 0, kt == KT - 1
                    nc.tensor.matmul(ps_q0, wq_sb[:, kt, 0:P], xt[:, kt, :],
                                     start=st, stop=sp)
                    nc.tensor.matmul(ps_q1, wq_sb[:, kt, P:2 * P], xt[:, kt, :],
                                     start=st, stop=sp)
                qr = qr_pool.tile([P, QH, NB], BF16, tag="qr", name="qr")
                rope(ps_q0, qr[:, 0, :], t0)
                rope(ps_q1, qr[:, 1, :], t0)
                emit_outproj_half(0)
                # pass B: k and v for the local kv head
                ps_k = pj.tile([P, NB], F32, tag="pj", name="ps_k")
                ps_v = pj.tile([P, NB], F32, tag="pj", name="ps_v")
                for kt in range(KT):
                    st, sp = kt == 0, kt == KT - 1
                    nc.tensor.matmul(ps_k, wk_sb[:, kt, :], xt[:, kt, :],
                                     start=st, stop=sp)
                    nc.tensor.matmul(ps_v, wv_sb[:, kt, :], xt[:, kt, :],
                                     start=st, stop=sp)
                rope(ps_k, kr_sb[:, b, ib * NB:(ib + 1) * NB], t0)
                vraw = raw_pool.tile([P, NB], BF16, tag="raw", name="vraw")
                nc.scalar.copy(vraw, ps_v)
                ps_tr = pj.tile([P, 4, P], BF16, tag="pj", name="ps_tr")
                for s4 in range(4):
                    nc.tensor.transpose(ps_tr[:, s4, :],
                                        vraw[:, s4 * P:(s4 + 1) * P], ident)
                nc.vector.tensor_copy(vt_sb[:, b, ib * 4:(ib + 1) * 4, :], ps_tr)
                emit_outproj_half(1)
                return qr

            def emit_attn(b, ib, qr):
                y_sb = y_pool.tile([P, QH, NB], BF16, tag="y", name="y_sb")
                njt = 4 * ib + 4
                for h in range(QH):
                    ps_y = py.tile([P, NB], F32, tag="py", name="ps_y")
                    acc = acc_pool.tile([P, NB], F32, tag="acc", name="acc")
                    for jt in range(njt):
                        a = jt - 4 * ib
                        sub = max(0, a) * P
                        ps = pst.tile([P, NB], F32, tag="st", name="ps_st")
                        nc.tensor.matmul(
                            ps[:, sub:],
                            kr_sb[:, b, jt * P:(jt + 1) * P],
                            qr[:, h, sub:],
                            start=True,
                            stop=True,
                        )
                        est = est_pool.tile([P, NB], BF16, tag="est", name="est")
                        nc.scalar.activation(est[:, sub:], ps[:, sub:], EXP,
                                             scale=SCALE)
                        if a >= 0:  # diagonal tile: causal triangle mask
                            nc.vector.tensor_mul(est[:, sub:sub + P],
                                                 est[:, sub:sub + P], triu)
                        if jt == 0:
                            nc.vector.tensor_copy(acc, est)
                        else:
                            nc.vector.tensor_add(acc[:, sub:], acc[:, sub:],
                                                 est[:, sub:])
                        nc.tensor.matmul(
                            ps_y[:, sub:],
                            vt_sb[:, b, jt, :],
                            est[:, sub:],
                            start=jt == 0,
                            stop=jt == njt - 1,
                        )
                    nc.gpsimd.partition_all_reduce(
                        acc, acc, channels=P, reduce_op=bass_isa.ReduceOp.add)
                    rinv = rinv_pool.tile([P, NB], F32, tag="rinv", name="rinv")
                    nc.vector.reciprocal_approx_fast(rinv, acc)
                    nc.vector.tensor_mul(y_sb[:, h, :], ps_y, rinv)
                return y_sb

            prefetch(0)
            prefetch(1)
            for b in range(B):
                for ib in range(IB):
                    gblk = b * IB + ib
                    qr = emit_proj(b, ib, gblk)
                    y_sb = emit_attn(b, ib, qr)
                    po_t = po.tile([P, 2, NB], F32, tag="po", name="po_t")
                    pending[0] = (b * T + ib * NB, y_sb, po_t)
            emit_outproj_half(0)
            emit_outproj_half(1)

    nc.compile()
    return nc


def _host_prep(x, rope, wq, wk, wv, wo):
    """Build the 8 per-core input maps (shard + pre-transpose + bf16)."""
    import ml_dtypes

    bf = ml_dtypes.bfloat16
    xT = np.ascontiguousarray(x.reshape(BT, D).T.astype(bf))
    cos = np.asarray(rope[..., 0], dtype=np.float32)  # [T, 64]
    sin = np.asarray(rope[..., 1], dtype=np.float32)
    cosT = np.concatenate([cos.T, cos.T], axis=0)  # [128, T]
    ssinT = np.concatenate([-sin.T, sin.T], axis=0)
    cf = np.ascontiguousarray(np.concatenate([cosT, ssinT], axis=1))
    permm = np.zeros((P, P), dtype=np.float32)
    permm[(np.arange(P) + 64) % P, np.arange(P)] = 1.0
    triu = np.triu(np.ones((P, P), dtype=np.float32))
    ident = np.eye(P, dtype=np.float32)
    cb = np.ascontiguousarray(
        np.concatenate([permm, triu, ident], axis=1).astype(bf))

    in_maps = []
    for c in range(NCORES):
        kv = c // 2
        in_maps.append(
            {
                "xT": xT,
                "wqT": np.ascontiguousarray(
                    wq[QH * HD * c:QH * HD * (c + 1), :].T.astype(bf)),
                "wkT": np.ascontiguousarray(
                    wk[HD * kv:HD * (kv + 1), :].T.astype(bf)),
                "wvT": np.ascontiguousarray(
                    wv[HD * kv:HD * (kv + 1), :].T.astype(bf)),
                "woT": np.ascontiguousarray(
                    wo[:, QH * HD * c:QH * HD * (c + 1)].T.astype(bf)),
                "cb": cb,
                "cf": cf,
            }
        )
    return in_maps


LAST_RESULTS = None


def kernel(x, rope, wq, wk, wv, wo):
    global LAST_RESULTS
    from concourse import bass_utils

    if "nc" not in _CACHE:
        _CACHE["nc"] = _build()
    nc = _CACHE["nc"]

    in_maps = _host_prep(
        np.asarray(x), np.asarray(rope), np.asarray(wq), np.asarray(wk),
        np.asarray(wv), np.asarray(wo)
    )
    res = bass_utils.run_bass_kernel_spmd(nc, in_maps, core_ids=list(range(NCORES)))
    LAST_RESULTS = res
    acc = np.zeros((BT, D), dtype=np.float64)
    for c in range(NCORES):
        acc += res.results[c]["out"].astype(np.float64)
    return acc.reshape(B, T, D).astype(np.float32)


# revision 4
# speedup vs baseline: 1.6886x; 1.4975x over previous
"""GQA attention forward on 8 TRN2 NeuronCores, tensor-parallel across heads.

Problem (hardcoded): B=2, T=2048, D=2048, 16 q-heads, 4 kv-heads, head_dim=128,
RoPE (rotate-half pairing i <-> i+64), causal softmax, output projection.

Sharding (per core c of 8):
  q-heads 2c, 2c+1 (rows 256c:256c+256 of wq), kv-head c//2 (rows of wk/wv),
  wo input-dim slice [:, 256c:256c+256]. x replicated. Each core computes a
  full-shape partial of the output (y_local @ wo_slice.T); host sums partials.

v3 design notes:
  - fp16 activations/weights (f32 PSUM): all tensors here are O(100) so fp16's
    4x-finer mantissa beats bf16 at identical PE/DMA cost, and 16-bit DVE ops
    run in 2x mode. CPU-sim rel err 7e-4 (max exp value ~1.1e3 << 65504).
  - Host packs every DRAM tensor partition-major so each DMA is 128 contiguous
    descriptors (the naive [D, features] layouts produced 256B descriptors
    that made weight loads 5-9us each).
  - Fused pipeline per 512-token block: proj -> attention, with the previous
    block's out-projection emitted one jb-chunk at a time BETWEEN attention
    j-tiles (attention alone is exp-throughput-bound on ACT at ~690ns/tile vs
    the PE's ~430ns/tile, so out-proj matmuls fill the PE bubbles).
  - Softmax denominator: est tiles accumulate elementwise into two fp16
    chains (even/odd j-tiles, halving the serial DVE latency), then ONE
    all-ones matmul per chain fuses the partition-reduce AND the broadcast
    (every output row = column sums) into a 512-cycle PE op. reciprocal via
    the approx-fast custom DVE op. No gpsimd in the chain (its library swaps
    between op types cost ~15us stalls in v2); gpsimd only runs the rope cos
    muls (single op type, single library).
  - PSUM = exactly 8 banks: pj(2: q0/q1/k/v/vtr ring), pst(2: rope-swap +
    score tiles + denom ring), py(2), po(2, jb ping-pong in halves).
"""
import math
import numpy as np

P = 128
B = 2
T = 2048
D = 2048
BT = B * T            # 4096
HD = 128              # head dim
QH = 2                # local q heads per core
KT = D // P           # 16 contraction tiles over D
NB = 512              # free-dim block (tokens)
IB = T // NB          # 4 i-blocks per batch
NJT_MAX = T // P      # 16 j-tiles per batch
NCORES = 8
SCALE = 1.0 / math.sqrt(HD)

_CACHE = {}


def _build():
    import concourse.bass as bass
    import concourse.mybir as mybir
    from concourse import bacc
    from concourse.tile import TileContext

    F32 = mybir.dt.float32
    F16 = mybir.dt.float16
    EXP = mybir.ActivationFunctionType.Exp

    nc = bacc.Bacc("TRN2", target_bir_lowering=False, debug=False)

    # all inputs partition-major-packed on host: [128, ...] contiguous rows
    x_d = nc.dram_tensor("xp", [P, 2 * IB * KT * NB], F16, kind="ExternalInput").ap()
    wqkv_d = nc.dram_tensor("wqkv", [P, KT * 4 * HD], F16, kind="ExternalInput").ap()
    wo_d = nc.dram_tensor("wop", [P, QH * D], F16, kind="ExternalInput").ap()
    cb_d = nc.dram_tensor("cb", [P, 4 * P], F16, kind="ExternalInput").ap()
    cf_d = nc.dram_tensor("cf", [P, 2 * T], F32, kind="ExternalInput").ap()
    out_d = nc.dram_tensor("out", [BT, D], F16, kind="ExternalOutput").ap()

    NHB = 2 * IB * 2   # 16 half-blocks of 8 kt-tiles each
    x_r = x_d.rearrange("p (hb kt m) -> p hb kt m", hb=NHB, kt=KT // 2)
    wqkv_r = wqkv_d.rearrange("p (h kt m) -> p h kt m", h=2, kt=KT // 2)
    wo_r = wo_d.rearrange("p (h j) -> p h j", h=QH)
    cb_r = cb_d.rearrange("p (a q) -> p a q", a=4)
    cf_r = cf_d.rearrange("p (a t) -> p a t", a=2)

    with TileContext(nc) as tc:
        with (
            tc.tile_pool(name="consts", bufs=1) as consts,
            tc.tile_pool(name="acts", bufs=1) as acts,
            tc.tile_pool(name="xt", bufs=4) as xt_pool,
            tc.tile_pool(name="qr", bufs=2) as qr_pool,
            tc.tile_pool(name="raw", bufs=3) as raw_pool,
            tc.tile_pool(name="tt", bufs=2) as t_pool,
            tc.tile_pool(name="est", bufs=4) as est_pool,
            tc.tile_pool(name="accp", bufs=4) as acc_pool,
            tc.tile_pool(name="rinv", bufs=2) as rinv_pool,
            tc.tile_pool(name="ysb", bufs=2) as y_pool,
            tc.tile_pool(name="osb", bufs=2) as o_pool,
            tc.tile_pool(name="pj", bufs=2, space="PSUM") as pj,
            tc.tile_pool(name="pst", bufs=2, space="PSUM") as pst,
            tc.tile_pool(name="py", bufs=2, space="PSUM") as py,
            tc.tile_pool(name="po", bufs=1, space="PSUM") as po,
        ):
            # ---- resident constants / weights (ACT HWDGE queue) ----
            cb_sb = consts.tile([P, 4, P], F16)
            wqkv_sb = consts.tile([P, 2, KT // 2, 4 * HD], F16)
            cs_sb = consts.tile([P, 2, T], F32)
            wo_sb = consts.tile([P, QH, D], F16)
            nc.scalar.dma_start(cb_sb, cb_r)
            nc.scalar.dma_start(wqkv_sb[:, 0], wqkv_r[:, 0])
            nc.scalar.dma_start(wqkv_sb[:, 1], wqkv_r[:, 1])
            nc.scalar.dma_start(cs_sb, cf_r)
            nc.scalar.dma_start(wo_sb, wo_r)
            perm = cb_sb[:, 0, :]
            triu = cb_sb[:, 1, :]
            ident = cb_sb[:, 2, :]
            aones = cb_sb[:, 3, :]
            cos_t = cs_sb[:, 0, :]
            sin_t = cs_sb[:, 1, :]

            def wslc(kt, c0, c1):
                return wqkv_sb[:, kt // 8, kt % 8, c0:c1]

            # ---- resident activations (per-batch slots) ----
            kr_sb = acts.tile([P, B, T], F16)
            vt_sb = acts.tile([P, B, NJT_MAX, HD], F16)

            xt_tiles = {}

            def prefetch(hb):
                if hb >= NHB or hb in xt_tiles:
                    return
                xt = xt_pool.tile([P, KT // 2, NB], F16, tag="xt", name="xt")
                nc.sync.dma_start(xt, x_r[:, hb])
                xt_tiles[hb] = xt

            def rope(ps_raw, dst, t0):
                # dst(fp16) = raw*cos + swap(raw)*ssin; swap via PE perm matmul
                raw = raw_pool.tile([P, NB], F16, tag="raw")
                nc.scalar.copy(raw, ps_raw)  # frees the psum bank quickly
                t1 = t_pool.tile([P, NB], F32, tag="t1")
                nc.gpsimd.tensor_mul(t1, raw, cos_t[:, t0:t0 + NB])
                ps_sw = pst.tile([P, NB], F32, tag="st", name="ps_sw")
                nc.tensor.matmul(ps_sw, perm, raw, start=True, stop=True)
                t2 = t_pool.tile([P, NB], F32, tag="t2")
                nc.vector.tensor_mul(t2, ps_sw, sin_t[:, t0:t0 + NB])
                nc.vector.tensor_add(dst, t1, t2)

            def make_outproj_steps(i0p, y_prev, po_t):
                steps = []
                state = {}

                def step(s, jb):
                    def run():
                        u = jb % 2
                        if jb == 0:
                            state["o"] = o_pool.tile([P, D], F16, tag="o",
                                                     name="o_sb")
                        o_sb = state["o"]
                        nc.tensor.matmul(
                            po_t[:, u, :],
                            y_prev[:, 0, s * P:(s + 1) * P],
                            wo_sb[:, 0, jb * NB:(jb + 1) * NB],
                            start=True, stop=False,
                        )
                        nc.tensor.matmul(
                            po_t[:, u, :],
                            y_prev[:, 1, s * P:(s + 1) * P],
                            wo_sb[:, 1, jb * NB:(jb + 1) * NB],
                            start=False, stop=True,
                        )
                        dst = o_sb[:, jb * NB:(jb + 1) * NB]
                        if jb == 1:  # 1-in-4 copies on ACT, rest on DVE
                            nc.scalar.copy(dst, po_t[:, u, :])
                        else:
                            nc.vector.tensor_copy(dst, po_t[:, u, :])
                        if jb == D // NB - 1:
                            row0 = i0p + s * P
                            nc.sync.dma_start(out_d[row0:row0 + P, :], o_sb)
                    return run

                for s in range(NB // P):
                    for jb in range(D // NB):
                        steps.append(step(s, jb))
                return steps

            def emit_proj(b, ib, gblk):
                xta = xt_tiles.pop(2 * gblk)
                xtb = xt_tiles.pop(2 * gblk + 1)
                prefetch(2 * gblk + 4)
                prefetch(2 * gblk + 5)
                t0 = ib * NB

                def xthalf(kt):
                    return (xta if kt < 8 else xtb)[:, kt % 8, :]

                # pass A: the two local q heads
                ps_q0 = pj.tile([P, NB], F32, tag="pj", name="ps_q0")
                ps_q1 = pj.tile([P, NB], F32, tag="pj", name="ps_q1")
                for kt in range(KT):
                    st, sp = kt == 0, kt == KT - 1
                    nc.tensor.matmul(ps_q0, wslc(kt, 0, P), xthalf(kt),
                                     start=st, stop=sp)
                    nc.tensor.matmul(ps_q1, wslc(kt, P, 2 * P), xthalf(kt),
                                     start=st, stop=sp)
                qr = qr_pool.tile([P, QH, NB], F16, tag="qr", name="qr")
                rope(ps_q0, qr[:, 0, :], t0)
                rope(ps_q1, qr[:, 1, :], t0)
                # pass B: k and v for the local kv head
                ps_k = pj.tile([P, NB], F32, tag="pj", name="ps_k")
                ps_v = pj.tile([P, NB], F32, tag="pj", name="ps_v")
                for kt in range(KT):
                    st, sp = kt == 0, kt == KT - 1
                    nc.tensor.matmul(ps_k, wslc(kt, 2 * P, 3 * P), xthalf(kt),
                                     start=st, stop=sp)
                    nc.tensor.matmul(ps_v, wslc(kt, 3 * P, 4 * P), xthalf(kt),
                                     start=st, stop=sp)
                rope(ps_k, kr_sb[:, b, ib * NB:(ib + 1) * NB], t0)
                vraw = raw_pool.tile([P, NB], F16, tag="raw", name="vraw")
                nc.scalar.copy(vraw, ps_v)
                ps_tr = pj.tile([P, 4, P], F16, tag="pj", name="ps_tr")
                for s4 in range(4):
                    nc.tensor.transpose(ps_tr[:, s4, :],
                                        vraw[:, s4 * P:(s4 + 1) * P], ident)
                nc.vector.tensor_copy(vt_sb[:, b, ib * 4:(ib + 1) * 4, :], ps_tr)
                return qr

            def emit_attn(b, ib, qr, steps):
                y_sb = y_pool.tile([P, QH, NB], F16, tag="y", name="y_sb")
                njt = 4 * ib + 4
                for h in range(QH):
                    ps_y = py.tile([P, NB], F32, tag="py", name="ps_y")
                    acc0 = acc_pool.tile([P, NB], F16, tag="acc", name="acc0")
                    acc1 = acc_pool.tile([P, NB], F16, tag="acc", name="acc1")
                    for jt in range(njt):
                        a = jt - 4 * ib
                        sub = max(0, a) * P
                        ps = pst.tile([P, NB], F32, tag="st", name="ps_st")
                        nc.tensor.matmul(
                            ps[:, sub:],
                            kr_sb[:, b, jt * P:(jt + 1) * P],
                            qr[:, h, sub:],
                            start=True, stop=True,
                        )
                        est = est_pool.tile([P, NB], F16, tag="est", name="est")
                        nc.scalar.activation(est[:, sub:], ps[:, sub:], EXP,
                                             scale=SCALE)
                        if a >= 0:  # diagonal tile: causal triangle mask
                            nc.vector.tensor_mul(est[:, sub:sub + P],
                                                 est[:, sub:sub + P], triu)
                        acc = acc0 if jt % 2 == 0 else acc1
                        if jt < 2:  # first tile of this chain
                            if sub > 0:
                                nc.vector.memset(acc[:, 0:sub], 0.0)
                            nc.vector.tensor_copy(acc[:, sub:], est[:, sub:])
                        else:
                            nc.vector.tensor_add(acc[:, sub:], acc[:, sub:],
                                                 est[:, sub:])
                        nc.tensor.matmul(
                            ps_y[:, sub:],
                            vt_sb[:, b, jt, :],
                            est[:, sub:],
                            start=jt == 0, stop=jt == njt - 1,
                        )
                        if steps:
                            steps.pop(0)()
                    # fused partition-reduce + broadcast: every row of the
                    # all-ones matmul output is the per-column denominator
                    rb_ps = pst.tile([P, NB], F32, tag="st", name="rb_ps")
                    nc.tensor.matmul(rb_ps, aones, acc0, start=True, stop=False)
                    nc.tensor.matmul(rb_ps, aones, acc1, start=False, stop=True)
                    rinv = rinv_pool.tile([P, NB], F32, tag="rinv", name="rinv")
                    nc.vector.reciprocal_approx_fast(rinv, rb_ps)
                    nc.vector.tensor_mul(y_sb[:, h, :], ps_y, rinv)
                return y_sb

            for hb in range(4):
                prefetch(hb)
            steps = []
            for b in range(B):
                for ib in range(IB):
                    gblk = b * IB + ib
                    qr = emit_proj(b, ib, gblk)
                    y_sb = emit_attn(b, ib, qr, steps)
                    for f in steps:  # leftovers (small-ib blocks)
                        f()
                    po_t = po.tile([P, 2, NB], F32, tag="po", name="po_t")
                    steps = make_outproj_steps(b * T + ib * NB, y_sb, po_t)
            for f in steps:
                f()

    nc.compile()
    return nc


def _host_prep(x, rope, wq, wk, wv, wo):
    """Build the 8 per-core input maps: shard, fp16, partition-major pack."""
    f16 = np.float16
    xT = x.reshape(BT, D).T.astype(f16)                 # [D, BT]
    xp = np.ascontiguousarray(
        xT.reshape(KT, P, 2 * IB, NB).transpose(1, 2, 0, 3).reshape(P, -1))
    cos = np.asarray(rope[..., 0], dtype=np.float32)    # [T, 64]
    sin = np.asarray(rope[..., 1], dtype=np.float32)
    cosT = np.concatenate([cos.T, cos.T], axis=0)       # [128, T]
    ssinT = np.concatenate([-sin.T, sin.T], axis=0)
    cf = np.ascontiguousarray(np.concatenate([cosT, ssinT], axis=1))
    permm = np.zeros((P, P), dtype=np.float32)
    permm[(np.arange(P) + 64) % P, np.arange(P)] = 1.0
    triu = np.triu(np.ones((P, P), dtype=np.float32))
    ident = np.eye(P, dtype=np.float32)
    aones = np.ones((P, P), dtype=np.float32)
    cb = np.ascontiguousarray(
        np.concatenate([permm, triu, ident, aones], axis=1).astype(f16))

    in_maps = []
    for c in range(NCORES):
        kv = c // 2
        wqkv = np.concatenate(
            [wq[QH * HD * c:QH * HD * (c + 1), :].T,
             wk[HD * kv:HD * (kv + 1), :].T,
             wv[HD * kv:HD * (kv + 1), :].T], axis=1).astype(f16)  # [D, 512]
        wqkv_p = np.ascontiguousarray(
            wqkv.reshape(KT, P, 4 * HD).transpose(1, 0, 2).reshape(P, -1))
        woT = wo[:, QH * HD * c:QH * HD * (c + 1)].T.astype(f16)   # [256, D]
        wo_p = np.ascontiguousarray(
            woT.reshape(QH, P, D).transpose(1, 0, 2).reshape(P, -1))
        in_maps.append(
            {"xp": xp, "wqkv": wqkv_p, "wop": wo_p, "cb": cb, "cf": cf}
        )
    return in_maps


LAST_RESULTS = None


def kernel(x, rope, wq, wk, wv, wo):
    global LAST_RESULTS
    from concourse import bass_utils

    if "nc" not in _CACHE:
        _CACHE["nc"] = _build()
    nc = _CACHE["nc"]

    in_maps = _host_prep(
        np.asarray(x), np.asarray(rope), np.asarray(wq), np.asarray(wk),
        np.asarray(wv), np.asarray(wo)
    )
    res = bass_utils.run_bass_kernel_spmd(nc, in_maps, core_ids=list(range(NCORES)))
    LAST_RESULTS = res
    acc = np.zeros((BT, D), dtype=np.float64)
    for c in range(NCORES):
        acc += res.results[c]["out"].astype(np.float64)
    return acc.reshape(B, T, D).astype(np.float32)
